# revision 37
# baseline (speedup 1.0000x reference)
"""DeepPoly ReLU backsubstitution kernel for Trainium2 (8 NeuronCores).

Math: the reference's sign-split matvecs reduce to two shared matvecs
    u1 = W @ c,  u2 = |W| @ r      (c = (ub+lb)/2, r = (ub-lb)/2 >= 0)
because both relu slopes are >= 0:
    new_ub = ub_slope*(u1 + u2 + b) + ub_bias
    new_lb = lb_slope*(u1 - u2 + b)
The 128 MB W traversal (memory-bound part) runs on 8 cores, data-parallel
over output rows; the O(N) slope/bias epilogue runs in numpy.

Sharding/layout: core k receives W[k*1024:(k+1)*1024].T reshaped to
[8, 128, 4096] — tile t, partition p holds W.T rows {t*512 + 4p + h},
h in [0,4), as four contiguous 1024-blocks along the free dim.  The
contraction dim j sits on SBUF partitions with no on-chip transpose and
each DMA moves one contiguous 2 MB slab.  Device pipeline per tile:
    DMA fp32 -> DVE fp32r-round copy (wt) + ACT |x| fp32r copy (at)
    -> 16 accumulating fp32r matvecs (full PE rate) -> psum u1/u2 -> out.
The lhsT vectors are host-permuted to match: crt col (t*4+h) = c[t*512+4p+h].
"""

import contextlib

import numpy as np

import concourse.bass as bass
import concourse.bacc as bacc
import concourse.tile as tile
from concourse import mybir
from concourse.bass_utils import run_bass_kernel_spmd

N = 8192
D = 4096
N_CORES = 8
ROWS = N // N_CORES          # 1024 output rows per core
N_TILE = 8                   # j-slabs per core (512 j each)
F32 = mybir.dt.float32
F32R = mybir.dt.float32r
AAbs = mybir.ActivationFunctionType.Abs
ACopy = mybir.ActivationFunctionType.Copy

_cached_nc = {}


def _build_nc(reps=1, variant="full", nat_bufs=4, wt_bufs=3, act_every=0,
              dma_eng="sync"):
    """variant: dma | full.  dma_eng: gpsimd | sync | mixed."""
    do_mm = variant == "full"
    nc = bacc.Bacc(None, target_bir_lowering=False)
    wt_dram = nc.dram_tensor("wt", [N_TILE, 128, 4096], F32, kind="ExternalInput")
    crt = nc.dram_tensor("crt", [128, 8 * N_TILE], F32, kind="ExternalInput")
    out = nc.dram_tensor("out", [2, ROWS], F32, kind="ExternalOutput")

    with tile.TileContext(nc) as tc:
        with (
            tc.tile_pool(name="const", bufs=1) as constp,
            tc.tile_pool(name="natw", bufs=nat_bufs) as natp,
            tc.tile_pool(name="wt", bufs=wt_bufs) as wtp,
            tc.tile_pool(name="at", bufs=wt_bufs) as atp,
            tc.tile_pool(name="osb", bufs=1) as osbp,
            tc.tile_pool(name="acc", bufs=1, space="PSUM") as accp,
        ):
            crt_f32 = constp.tile([128, 8 * N_TILE], F32, tag="crtf")
            nc.gpsimd.dma_start(crt_f32[:], crt[:])
            # fp32r-rounded copy: required producer for fp32r matmul lhsT
            crt_sb = constp.tile([128, 8 * N_TILE], F32R, tag="crt")
            nc.vector.tensor_copy(crt_sb[:], crt_f32[:])

            rep_ctx = tc.For_i(0, reps, 1) if reps > 1 else contextlib.nullcontext()
            with rep_ctx:
                u1_sb = osbp.tile([1, ROWS], F32, tag="u1sb")
                u2_sb = osbp.tile([1, ROWS], F32, tag="u2sb")

                if do_mm:
                    ps_u1a = accp.tile([1, 512], F32, tag="u1a")
                    ps_u1b = accp.tile([1, 512], F32, tag="u1b")
                    ps_u2a = accp.tile([1, 512], F32, tag="u2a")
                    ps_u2b = accp.tile([1, 512], F32, tag="u2b")
                    ps_u1 = [ps_u1a, ps_u1b]
                    ps_u2 = [ps_u2a, ps_u2b]

                for t in range(N_TILE):
                    nat = natp.tile([128, 4096], F32, tag="nat")
                    if dma_eng == "gpsimd":
                        eng = nc.gpsimd
                    elif dma_eng == "sync":
                        eng = nc.sync
                    else:
                        eng = nc.sync if t % 2 == 0 else nc.scalar
                    eng.dma_start(nat[:], wt_dram[t])
                    if not do_mm:
                        if t == 0:
                            nc.vector.tensor_copy(u1_sb[:], nat[0:1, 0:ROWS])
                            nc.vector.tensor_copy(u2_sb[:], nat[0:1, 0:ROWS])
                        continue
                    wt_t = wtp.tile([128, 4096], F32R, tag="wt")
                    at_t = atp.tile([128, 4096], F32R, tag="at")
                    for h in range(4):
                        if h % 2 == 0:
                            qsl = slice(h * 1024, (h + 2) * 1024)
                            nc.vector.tensor_copy(wt_t[:, qsl], nat[:, qsl])
                            nc.scalar.activation(at_t[:, qsl], nat[:, qsl], AAbs)
                        col = t * 4 + h
                        st = t == 0 and h == 0
                        sp = t == N_TILE - 1 and h == 3
                        for half in range(2):
                            sl = slice(h * 1024 + half * 512, h * 1024 + (half + 1) * 512)
                            nc.tensor.matmul(
                                ps_u1[half][:],
                                lhsT=crt_sb[:, col : col + 1],
                                rhs=wt_t[:, sl],
                                start=st, stop=sp,
                            )
                            nc.tensor.matmul(
                                ps_u2[half][:],
                                lhsT=crt_sb[:, 32 + col : 32 + col + 1],
                                rhs=at_t[:, sl],
                                start=st, stop=sp,
                            )

                if do_mm:
                    for half in range(2):
                        sl = slice(half * 512, (half + 1) * 512)
                        nc.vector.tensor_copy(u1_sb[:, sl], ps_u1[half][:])
                        nc.vector.tensor_copy(u2_sb[:, sl], ps_u2[half][:])

                nc.gpsimd.dma_start(out[0:1, :], u1_sb[:])
                nc.gpsimd.dma_start(out[1:2, :], u2_sb[:])

    nc.compile()
    return nc


def _get_nc(reps=1, **kw):
    key = (reps, tuple(sorted(kw.items())))
    if key not in _cached_nc:
        _cached_nc[key] = _build_nc(reps, **kw)
    return _cached_nc[key]


def _prep_in_maps(W, orig_ub, orig_lb):
    c = ((orig_ub + orig_lb) * np.float32(0.5)).astype(np.float32)
    r = ((orig_ub - orig_lb) * np.float32(0.5)).astype(np.float32)
    # crt col (t*4+h)[p] = vec[t*512 + 4p + h]
    cperm = np.ascontiguousarray(
        c.reshape(N_TILE, 128, 4).transpose(1, 0, 2).reshape(128, 32)
    )
    rperm = np.ascontiguousarray(
        r.reshape(N_TILE, 128, 4).transpose(1, 0, 2).reshape(128, 32)
    )
    crt = np.ascontiguousarray(np.concatenate([cperm, rperm], axis=1)).astype(
        np.float32
    )
    return [
        {
            "wt": np.ascontiguousarray(
                W[k * ROWS : (k + 1) * ROWS].T
            ).reshape(N_TILE, 128, 4096),
            "crt": crt,
        }
        for k in range(N_CORES)
    ]


def kernel(orig_ub, orig_lb, prev_ub, prev_lb, alpha, W, b):
    orig_ub = np.asarray(orig_ub, dtype=np.float32)
    orig_lb = np.asarray(orig_lb, dtype=np.float32)
    prev_ub = np.asarray(prev_ub, dtype=np.float32)
    prev_lb = np.asarray(prev_lb, dtype=np.float32)
    alpha = np.asarray(alpha, dtype=np.float32)
    W = np.asarray(W, dtype=np.float32)
    b = np.asarray(b, dtype=np.float32)

    in_maps = _prep_in_maps(W, orig_ub, orig_lb)
    res = run_bass_kernel_spmd(_get_nc(), in_maps, list(range(N_CORES)))
    u1 = np.concatenate([res.results[k]["out"][0] for k in range(N_CORES)])
    u2 = np.concatenate([res.results[k]["out"][1] for k in range(N_CORES)])

    # epilogue: identical mask logic to the reference, in fp32 numpy
    neg = prev_ub <= 0.0
    cross = (prev_ub > 0.0) & (prev_lb < 0.0)
    denom = np.where(cross, prev_ub - prev_lb, np.float32(1.0)).astype(np.float32)
    ub_slope = np.where(
        cross, prev_ub / denom, np.where(neg, np.float32(0.0), np.float32(1.0))
    ).astype(np.float32)
    lb_slope = np.where(
        cross, alpha, np.where(neg, np.float32(0.0), np.float32(1.0))
    ).astype(np.float32)
    ub_bias = np.where(cross, -ub_slope * prev_lb, np.float32(0.0)).astype(np.float32)

    new_ub = ub_slope * (u1 + u2 + b) + ub_bias
    new_lb = lb_slope * (u1 - u2 + b)
    return np.stack([new_ub, new_lb]).astype(np.float32)


# revision 38
# speedup vs baseline: 1.0432x; 1.0432x over previous
"""DeepPoly ReLU backsubstitution kernel for Trainium2 (8 NeuronCores).

Math: the reference's sign-split matvecs reduce to two shared matvecs
    u1 = W @ c,  u2 = |W| @ r      (c = (ub+lb)/2, r = (ub-lb)/2 >= 0)
because both relu slopes are >= 0:
    new_ub = ub_slope*(u1 + u2 + b) + ub_bias
    new_lb = lb_slope*(u1 - u2 + b)
The 128 MB W traversal (memory-bound part) runs on 8 cores, data-parallel
over output rows; the O(N) slope/bias epilogue runs in numpy.

Sharding/layout: core k receives W[k*1024:(k+1)*1024].T reshaped to
[8, 128, 4096] — tile t, partition p holds W.T rows {t*512 + 4p + h},
h in [0,4), as four contiguous 1024-blocks along the free dim.  The
contraction dim j sits on SBUF partitions with no on-chip transpose and
each DMA moves one contiguous 2 MB slab.  Device pipeline per tile:
    DMA fp32 -> DVE fp32r-round copy (wt) + ACT |x| fp32r copy (at)
    -> 16 accumulating fp32r matvecs (full PE rate) -> psum u1/u2 -> out.
The lhsT vectors are host-permuted to match: crt col (t*4+h) = c[t*512+4p+h].
"""

import contextlib

import numpy as np

import concourse.bass as bass
import concourse.bacc as bacc
import concourse.tile as tile
from concourse import mybir
from concourse.bass_utils import run_bass_kernel_spmd

N = 8192
D = 4096
N_CORES = 8
ROWS = N // N_CORES          # 1024 output rows per core
N_TILE = 8                   # j-slabs per core (512 j each)
F32 = mybir.dt.float32
F32R = mybir.dt.float32r
AAbs = mybir.ActivationFunctionType.Abs
ACopy = mybir.ActivationFunctionType.Copy

_cached_nc = {}


def _build_nc(reps=1, variant="full", nat_bufs=4, wt_bufs=3, act_every=0,
              dma_eng="sync"):
    """variant: dma | full.  dma_eng: gpsimd | sync | mixed."""
    do_mm = variant == "full"
    nc = bacc.Bacc(None, target_bir_lowering=False)
    wt_dram = nc.dram_tensor("wt", [N_TILE, 128, 4096], F32, kind="ExternalInput")
    crt = nc.dram_tensor("crt", [128, 8 * N_TILE], F32, kind="ExternalInput")
    out = nc.dram_tensor("out", [2, ROWS], F32, kind="ExternalOutput")

    with tile.TileContext(nc) as tc:
        with (
            tc.tile_pool(name="const", bufs=1) as constp,
            tc.tile_pool(name="natw", bufs=nat_bufs) as natp,
            tc.tile_pool(name="wt", bufs=wt_bufs) as wtp,
            tc.tile_pool(name="at", bufs=wt_bufs) as atp,
            tc.tile_pool(name="osb", bufs=1) as osbp,
            tc.tile_pool(name="acc", bufs=1, space="PSUM") as accp,
        ):
            crt_f32 = constp.tile([128, 8 * N_TILE], F32, tag="crtf")
            nc.gpsimd.dma_start(crt_f32[:], crt[:])
            # fp32r-rounded copy: required producer for fp32r matmul lhsT
            crt_sb = constp.tile([128, 8 * N_TILE], F32R, tag="crt")
            nc.vector.tensor_copy(crt_sb[:], crt_f32[:])

            rep_ctx = tc.For_i(0, reps, 1) if reps > 1 else contextlib.nullcontext()
            with rep_ctx:
                u1_sb = osbp.tile([1, ROWS], F32, tag="u1sb")
                u2_sb = osbp.tile([1, ROWS], F32, tag="u2sb")

                if do_mm:
                    ps_u1a = accp.tile([1, 512], F32, tag="u1a")
                    ps_u1b = accp.tile([1, 512], F32, tag="u1b")
                    ps_u2a = accp.tile([1, 512], F32, tag="u2a")
                    ps_u2b = accp.tile([1, 512], F32, tag="u2b")
                    ps_u1 = [ps_u1a, ps_u1b]
                    ps_u2 = [ps_u2a, ps_u2b]

                for t in range(N_TILE):
                    nat = natp.tile([128, 4096], F32, tag="nat")
                    if dma_eng == "gpsimd":
                        eng = nc.gpsimd
                    elif dma_eng == "sync":
                        eng = nc.sync
                    else:
                        eng = nc.sync if t % 2 == 0 else nc.scalar
                    eng.dma_start(nat[:], wt_dram[t])
                    if not do_mm:
                        if t == 0:
                            nc.vector.tensor_copy(u1_sb[:], nat[0:1, 0:ROWS])
                            nc.vector.tensor_copy(u2_sb[:], nat[0:1, 0:ROWS])
                        continue
                    wt_t = wtp.tile([128, 4096], F32R, tag="wt")
                    at_t = atp.tile([128, 4096], F32R, tag="at")
                    nc.vector.tensor_copy(wt_t[:], nat[:])
                    nc.scalar.activation(at_t[:], nat[:], AAbs)
                    for h in range(4):
                        col = t * 4 + h
                        st = t == 0 and h == 0
                        sp = t == N_TILE - 1 and h == 3
                        for half in range(2):
                            sl = slice(h * 1024 + half * 512, h * 1024 + (half + 1) * 512)
                            nc.tensor.matmul(
                                ps_u1[half][:],
                                lhsT=crt_sb[:, col : col + 1],
                                rhs=wt_t[:, sl],
                                start=st, stop=sp,
                            )
                            nc.tensor.matmul(
                                ps_u2[half][:],
                                lhsT=crt_sb[:, 32 + col : 32 + col + 1],
                                rhs=at_t[:, sl],
                                start=st, stop=sp,
                            )

                if do_mm:
                    for half in range(2):
                        sl = slice(half * 512, (half + 1) * 512)
                        nc.vector.tensor_copy(u1_sb[:, sl], ps_u1[half][:])
                        nc.vector.tensor_copy(u2_sb[:, sl], ps_u2[half][:])

                nc.gpsimd.dma_start(out[0:1, :], u1_sb[:])
                nc.gpsimd.dma_start(out[1:2, :], u2_sb[:])

    nc.compile()
    return nc


def _get_nc(reps=1, **kw):
    key = (reps, tuple(sorted(kw.items())))
    if key not in _cached_nc:
        _cached_nc[key] = _build_nc(reps, **kw)
    return _cached_nc[key]


def _prep_in_maps(W, orig_ub, orig_lb):
    c = ((orig_ub + orig_lb) * np.float32(0.5)).astype(np.float32)
    r = ((orig_ub - orig_lb) * np.float32(0.5)).astype(np.float32)
    # crt col (t*4+h)[p] = vec[t*512 + 4p + h]
    cperm = np.ascontiguousarray(
        c.reshape(N_TILE, 128, 4).transpose(1, 0, 2).reshape(128, 32)
    )
    rperm = np.ascontiguousarray(
        r.reshape(N_TILE, 128, 4).transpose(1, 0, 2).reshape(128, 32)
    )
    crt = np.ascontiguousarray(np.concatenate([cperm, rperm], axis=1)).astype(
        np.float32
    )
    return [
        {
            "wt": np.ascontiguousarray(
                W[k * ROWS : (k + 1) * ROWS].T
            ).reshape(N_TILE, 128, 4096),
            "crt": crt,
        }
        for k in range(N_CORES)
    ]


def kernel(orig_ub, orig_lb, prev_ub, prev_lb, alpha, W, b):
    orig_ub = np.asarray(orig_ub, dtype=np.float32)
    orig_lb = np.asarray(orig_lb, dtype=np.float32)
    prev_ub = np.asarray(prev_ub, dtype=np.float32)
    prev_lb = np.asarray(prev_lb, dtype=np.float32)
    alpha = np.asarray(alpha, dtype=np.float32)
    W = np.asarray(W, dtype=np.float32)
    b = np.asarray(b, dtype=np.float32)

    in_maps = _prep_in_maps(W, orig_ub, orig_lb)
    res = run_bass_kernel_spmd(_get_nc(), in_maps, list(range(N_CORES)))
    u1 = np.concatenate([res.results[k]["out"][0] for k in range(N_CORES)])
    u2 = np.concatenate([res.results[k]["out"][1] for k in range(N_CORES)])

    # epilogue: identical mask logic to the reference, in fp32 numpy
    neg = prev_ub <= 0.0
    cross = (prev_ub > 0.0) & (prev_lb < 0.0)
    denom = np.where(cross, prev_ub - prev_lb, np.float32(1.0)).astype(np.float32)
    ub_slope = np.where(
        cross, prev_ub / denom, np.where(neg, np.float32(0.0), np.float32(1.0))
    ).astype(np.float32)
    lb_slope = np.where(
        cross, alpha, np.where(neg, np.float32(0.0), np.float32(1.0))
    ).astype(np.float32)
    ub_bias = np.where(cross, -ub_slope * prev_lb, np.float32(0.0)).astype(np.float32)

    new_ub = ub_slope * (u1 + u2 + b) + ub_bias
    new_lb = lb_slope * (u1 - u2 + b)
    return np.stack([new_ub, new_lb]).astype(np.float32)


# revision 40
# speedup vs baseline: 1.0644x; 1.0203x over previous
"""DeepPoly ReLU backsubstitution kernel for Trainium2 (8 NeuronCores).

Math: the reference's sign-split matvecs reduce to two shared matvecs
    u1 = W @ c,  u2 = |W| @ r      (c = (ub+lb)/2, r = (ub-lb)/2 >= 0)
because both relu slopes are >= 0:
    new_ub = ub_slope*(u1 + u2 + b) + ub_bias
    new_lb = lb_slope*(u1 - u2 + b)
The 128 MB W traversal (memory-bound part) runs on 8 cores, data-parallel
over output rows; the O(N) slope/bias epilogue runs in numpy.

Sharding/layout: core k receives W[k*1024:(k+1)*1024].T reshaped to
[8, 128, 4096] — tile t, partition p holds W.T rows {t*512 + 4p + h},
h in [0,4), as four contiguous 1024-blocks along the free dim.  The
contraction dim j sits on SBUF partitions with no on-chip transpose and
each DMA moves one contiguous 2 MB slab.  Device pipeline per tile:
    DMA fp32 -> DVE fp32r-round copy (wt) + ACT |x| fp32r copy (at)
    -> 16 accumulating fp32r matvecs (full PE rate) -> psum u1/u2 -> out.
The lhsT vectors are host-permuted to match: crt col (t*4+h) = c[t*512+4p+h].
"""

import contextlib

import numpy as np

import concourse.bass as bass
import concourse.bacc as bacc
import concourse.tile as tile
from concourse import mybir
from concourse.bass_utils import run_bass_kernel_spmd

N = 8192
D = 4096
N_CORES = 8
ROWS = N // N_CORES          # 1024 output rows per core
N_TILE = 8                   # j-slabs per core (512 j each)
F32 = mybir.dt.float32
F32R = mybir.dt.float32r
AAbs = mybir.ActivationFunctionType.Abs
ACopy = mybir.ActivationFunctionType.Copy

_cached_nc = {}


def _build_nc(reps=1, variant="full", nat_bufs=4, wt_bufs=3, act_every=0,
              dma_eng="sync"):
    """variant: dma | full.  dma_eng: gpsimd | sync | mixed."""
    do_mm = variant == "full"
    nc = bacc.Bacc(None, target_bir_lowering=False)
    wt_dram = nc.dram_tensor("wt", [N_TILE, 128, 4096], F32, kind="ExternalInput")
    crt = nc.dram_tensor("crt", [128, 8 * N_TILE], F32, kind="ExternalInput")
    out = nc.dram_tensor("out", [2, ROWS], F32, kind="ExternalOutput")

    with tile.TileContext(nc) as tc:
        with (
            tc.tile_pool(name="const", bufs=1) as constp,
            tc.tile_pool(name="natw", bufs=nat_bufs) as natp,
            tc.tile_pool(name="wt", bufs=wt_bufs) as wtp,
            tc.tile_pool(name="at", bufs=wt_bufs) as atp,
            tc.tile_pool(name="osb", bufs=1) as osbp,
            tc.tile_pool(name="acc", bufs=1, space="PSUM") as accp,
        ):
            crt_f32 = constp.tile([128, 8 * N_TILE], F32, tag="crtf")
            nc.gpsimd.dma_start(crt_f32[:], crt[:])
            # fp32r-rounded copy: required producer for fp32r matmul lhsT
            crt_sb = constp.tile([128, 8 * N_TILE], F32R, tag="crt")
            nc.vector.tensor_copy(crt_sb[:], crt_f32[:])

            rep_ctx = tc.For_i(0, reps, 1) if reps > 1 else contextlib.nullcontext()
            with rep_ctx:
                u1_sb = osbp.tile([1, ROWS], F32, tag="u1sb")
                u2_sb = osbp.tile([1, ROWS], F32, tag="u2sb")

                if do_mm:
                    ps_u1a = accp.tile([1, 512], F32, tag="u1a")
                    ps_u1b = accp.tile([1, 512], F32, tag="u1b")
                    ps_u2a = accp.tile([1, 512], F32, tag="u2a")
                    ps_u2b = accp.tile([1, 512], F32, tag="u2b")
                    ps_u1 = [ps_u1a, ps_u1b]
                    ps_u2 = [ps_u2a, ps_u2b]

                for t in range(N_TILE):
                    split = t in (0, N_TILE - 1)
                    nat = natp.tile([128, 4096], F32, tag="nat")
                    if dma_eng == "gpsimd":
                        eng = nc.gpsimd
                    elif dma_eng == "sync":
                        eng = nc.sync
                    else:
                        eng = nc.sync if t % 2 == 0 else nc.scalar
                    if split:
                        for h in range(4):
                            qsl = slice(h * 1024, (h + 1) * 1024)
                            eng.dma_start(nat[:, qsl], wt_dram[t][:, qsl])
                    else:
                        eng.dma_start(nat[:], wt_dram[t])
                    if not do_mm:
                        if t == 0:
                            nc.vector.tensor_copy(u1_sb[:], nat[0:1, 0:ROWS])
                            nc.vector.tensor_copy(u2_sb[:], nat[0:1, 0:ROWS])
                        continue
                    wt_t = wtp.tile([128, 4096], F32R, tag="wt")
                    at_t = atp.tile([128, 4096], F32R, tag="at")
                    if not split:
                        nc.vector.tensor_copy(wt_t[:], nat[:])
                        nc.scalar.activation(at_t[:], nat[:], AAbs)
                    for h in range(4):
                        if split:
                            qsl = slice(h * 1024, (h + 1) * 1024)
                            nc.vector.tensor_copy(wt_t[:, qsl], nat[:, qsl])
                            nc.scalar.activation(at_t[:, qsl], nat[:, qsl], AAbs)
                        col = t * 4 + h
                        st = t == 0 and h == 0
                        sp = t == N_TILE - 1 and h == 3
                        for half in range(2):
                            sl = slice(h * 1024 + half * 512, h * 1024 + (half + 1) * 512)
                            nc.tensor.matmul(
                                ps_u1[half][:],
                                lhsT=crt_sb[:, col : col + 1],
                                rhs=wt_t[:, sl],
                                start=st, stop=sp,
                            )
                            nc.tensor.matmul(
                                ps_u2[half][:],
                                lhsT=crt_sb[:, 32 + col : 32 + col + 1],
                                rhs=at_t[:, sl],
                                start=st, stop=sp,
                            )

                if do_mm:
                    # drain accumulators on ACT and DVE in parallel
                    nc.scalar.activation(u1_sb[:, 0:512], ps_u1[0][:], ACopy)
                    nc.vector.tensor_copy(u1_sb[:, 512:1024], ps_u1[1][:])
                    nc.scalar.activation(u2_sb[:, 0:512], ps_u2[0][:], ACopy)
                    nc.vector.tensor_copy(u2_sb[:, 512:1024], ps_u2[1][:])

                nc.sync.dma_start(out[0:1, :], u1_sb[:])
                nc.sync.dma_start(out[1:2, :], u2_sb[:])

    nc.compile()
    return nc


def _get_nc(reps=1, **kw):
    key = (reps, tuple(sorted(kw.items())))
    if key not in _cached_nc:
        _cached_nc[key] = _build_nc(reps, **kw)
    return _cached_nc[key]


def _prep_in_maps(W, orig_ub, orig_lb):
    c = ((orig_ub + orig_lb) * np.float32(0.5)).astype(np.float32)
    r = ((orig_ub - orig_lb) * np.float32(0.5)).astype(np.float32)
    # crt col (t*4+h)[p] = vec[t*512 + 4p + h]
    cperm = np.ascontiguousarray(
        c.reshape(N_TILE, 128, 4).transpose(1, 0, 2).reshape(128, 32)
    )
    rperm = np.ascontiguousarray(
        r.reshape(N_TILE, 128, 4).transpose(1, 0, 2).reshape(128, 32)
    )
    crt = np.ascontiguousarray(np.concatenate([cperm, rperm], axis=1)).astype(
        np.float32
    )
    return [
        {
            "wt": np.ascontiguousarray(
                W[k * ROWS : (k + 1) * ROWS].T
            ).reshape(N_TILE, 128, 4096),
            "crt": crt,
        }
        for k in range(N_CORES)
    ]


def kernel(orig_ub, orig_lb, prev_ub, prev_lb, alpha, W, b):
    orig_ub = np.asarray(orig_ub, dtype=np.float32)
    orig_lb = np.asarray(orig_lb, dtype=np.float32)
    prev_ub = np.asarray(prev_ub, dtype=np.float32)
    prev_lb = np.asarray(prev_lb, dtype=np.float32)
    alpha = np.asarray(alpha, dtype=np.float32)
    W = np.asarray(W, dtype=np.float32)
    b = np.asarray(b, dtype=np.float32)

    in_maps = _prep_in_maps(W, orig_ub, orig_lb)
    res = run_bass_kernel_spmd(_get_nc(), in_maps, list(range(N_CORES)))
    u1 = np.concatenate([res.results[k]["out"][0] for k in range(N_CORES)])
    u2 = np.concatenate([res.results[k]["out"][1] for k in range(N_CORES)])

    # epilogue: identical mask logic to the reference, in fp32 numpy
    neg = prev_ub <= 0.0
    cross = (prev_ub > 0.0) & (prev_lb < 0.0)
    denom = np.where(cross, prev_ub - prev_lb, np.float32(1.0)).astype(np.float32)
    ub_slope = np.where(
        cross, prev_ub / denom, np.where(neg, np.float32(0.0), np.float32(1.0))
    ).astype(np.float32)
    lb_slope = np.where(
        cross, alpha, np.where(neg, np.float32(0.0), np.float32(1.0))
    ).astype(np.float32)
    ub_bias = np.where(cross, -ub_slope * prev_lb, np.float32(0.0)).astype(np.float32)

    new_ub = ub_slope * (u1 + u2 + b) + ub_bias
    new_lb = lb_slope * (u1 - u2 + b)
    return np.stack([new_ub, new_lb]).astype(np.float32)


# revision 41
# speedup vs baseline: 1.0838x; 1.0183x over previous
"""DeepPoly ReLU backsubstitution kernel for Trainium2 (8 NeuronCores).

Math: the reference's sign-split matvecs reduce to two shared matvecs
    u1 = W @ c,  u2 = |W| @ r      (c = (ub+lb)/2, r = (ub-lb)/2 >= 0)
because both relu slopes are >= 0:
    new_ub = ub_slope*(u1 + u2 + b) + ub_bias
    new_lb = lb_slope*(u1 - u2 + b)
The 128 MB W traversal (memory-bound part) runs on 8 cores, data-parallel
over output rows; the O(N) slope/bias epilogue runs in numpy.

Sharding/layout: core k receives W[k*1024:(k+1)*1024].T reshaped to
[8, 128, 4096] — tile t, partition p holds W.T rows {t*512 + 4p + h},
h in [0,4), as four contiguous 1024-blocks along the free dim.  The
contraction dim j sits on SBUF partitions with no on-chip transpose and
each DMA moves one contiguous 2 MB slab.  Device pipeline per tile:
    DMA fp32 -> DVE fp32r-round copy (wt) + ACT |x| fp32r copy (at)
    -> 16 accumulating fp32r matvecs (full PE rate) -> psum u1/u2 -> out.
The lhsT vectors are host-permuted to match: crt col (t*4+h) = c[t*512+4p+h].
"""

import contextlib

import numpy as np

import concourse.bass as bass
import concourse.bacc as bacc
import concourse.tile as tile
from concourse import mybir
from concourse.bass_utils import run_bass_kernel_spmd

N = 8192
D = 4096
N_CORES = 8
ROWS = N // N_CORES          # 1024 output rows per core
N_TILE = 8                   # j-slabs per core (512 j each)
F32 = mybir.dt.float32
F32R = mybir.dt.float32r
AAbs = mybir.ActivationFunctionType.Abs
ACopy = mybir.ActivationFunctionType.Copy

_cached_nc = {}


def _build_nc(reps=1, variant="full", nat_bufs=4, wt_bufs=3, act_every=0,
              dma_eng="sync"):
    """variant: dma | full.  dma_eng: gpsimd | sync | mixed."""
    do_mm = variant == "full"
    nc = bacc.Bacc(None, target_bir_lowering=False)
    wt_dram = nc.dram_tensor("wt", [N_TILE, 128, 4096], F32, kind="ExternalInput")
    crt = nc.dram_tensor("crt", [128, 8 * N_TILE], F32, kind="ExternalInput")
    out = nc.dram_tensor("out", [2, ROWS], F32, kind="ExternalOutput")

    with tile.TileContext(nc) as tc:
        with (
            tc.tile_pool(name="const", bufs=1) as constp,
            tc.tile_pool(name="natw", bufs=nat_bufs) as natp,
            tc.tile_pool(name="wt", bufs=wt_bufs) as wtp,
            tc.tile_pool(name="at", bufs=wt_bufs) as atp,
            tc.tile_pool(name="osb", bufs=1) as osbp,
            tc.tile_pool(name="acc", bufs=1, space="PSUM") as accp,
        ):
            crt_f32 = constp.tile([128, 8 * N_TILE], F32, tag="crtf")
            nc.gpsimd.dma_start(crt_f32[:], crt[:])
            # fp32r-rounded copy: required producer for fp32r matmul lhsT
            crt_sb = constp.tile([128, 8 * N_TILE], F32R, tag="crt")
            nc.vector.tensor_copy(crt_sb[:], crt_f32[:])

            rep_ctx = (
                tc.For_i(0, reps, 1, hint_engines=(mybir.EngineType.PE,))
                if reps > 1
                else contextlib.nullcontext()
            )
            with rep_ctx:
                u1_sb = osbp.tile([1, ROWS], F32, tag="u1sb")
                u2_sb = osbp.tile([1, ROWS], F32, tag="u2sb")

                if do_mm:
                    ps_u1a = accp.tile([1, 512], F32, tag="u1a")
                    ps_u1b = accp.tile([1, 512], F32, tag="u1b")
                    ps_u2a = accp.tile([1, 512], F32, tag="u2a")
                    ps_u2b = accp.tile([1, 512], F32, tag="u2b")
                    ps_u1 = [ps_u1a, ps_u1b]
                    ps_u2 = [ps_u2a, ps_u2b]

                for t in range(N_TILE):
                    split = t in (0, N_TILE - 1)
                    nat = natp.tile([128, 4096], F32, tag="nat")
                    if dma_eng == "gpsimd":
                        eng = nc.gpsimd
                    elif dma_eng == "sync":
                        eng = nc.sync
                    else:
                        eng = nc.sync if t % 2 == 0 else nc.scalar
                    if split:
                        for h in range(4):
                            qsl = slice(h * 1024, (h + 1) * 1024)
                            eng.dma_start(nat[:, qsl], wt_dram[t][:, qsl])
                    else:
                        eng.dma_start(nat[:], wt_dram[t])
                    if not do_mm:
                        if t == 0:
                            nc.vector.tensor_copy(u1_sb[:], nat[0:1, 0:ROWS])
                            nc.vector.tensor_copy(u2_sb[:], nat[0:1, 0:ROWS])
                        continue
                    wt_t = wtp.tile([128, 4096], F32R, tag="wt")
                    at_t = atp.tile([128, 4096], F32R, tag="at")
                    if not split:
                        nc.vector.tensor_copy(wt_t[:], nat[:])
                        nc.scalar.activation(at_t[:], nat[:], AAbs)
                    for h in range(4):
                        if split:
                            qsl = slice(h * 1024, (h + 1) * 1024)
                            nc.vector.tensor_copy(wt_t[:, qsl], nat[:, qsl])
                            nc.scalar.activation(at_t[:, qsl], nat[:, qsl], AAbs)
                        col = t * 4 + h
                        st = t == 0 and h == 0
                        sp = t == N_TILE - 1 and h == 3
                        for half in range(2):
                            sl = slice(h * 1024 + half * 512, h * 1024 + (half + 1) * 512)
                            nc.tensor.matmul(
                                ps_u1[half][:],
                                lhsT=crt_sb[:, col : col + 1],
                                rhs=wt_t[:, sl],
                                start=st, stop=sp,
                            )
                            nc.tensor.matmul(
                                ps_u2[half][:],
                                lhsT=crt_sb[:, 32 + col : 32 + col + 1],
                                rhs=at_t[:, sl],
                                start=st, stop=sp,
                            )

                if do_mm:
                    # drain accumulators on ACT and DVE in parallel
                    nc.scalar.activation(u1_sb[:, 0:512], ps_u1[0][:], ACopy)
                    nc.vector.tensor_copy(u1_sb[:, 512:1024], ps_u1[1][:])
                    nc.scalar.activation(u2_sb[:, 0:512], ps_u2[0][:], ACopy)
                    nc.vector.tensor_copy(u2_sb[:, 512:1024], ps_u2[1][:])

                nc.sync.dma_start(out[0:1, :], u1_sb[:])
                nc.sync.dma_start(out[1:2, :], u2_sb[:])

    nc.compile()
    return nc


def _get_nc(reps=1, **kw):
    key = (reps, tuple(sorted(kw.items())))
    if key not in _cached_nc:
        _cached_nc[key] = _build_nc(reps, **kw)
    return _cached_nc[key]


def _prep_in_maps(W, orig_ub, orig_lb):
    c = ((orig_ub + orig_lb) * np.float32(0.5)).astype(np.float32)
    r = ((orig_ub - orig_lb) * np.float32(0.5)).astype(np.float32)
    # crt col (t*4+h)[p] = vec[t*512 + 4p + h]
    cperm = np.ascontiguousarray(
        c.reshape(N_TILE, 128, 4).transpose(1, 0, 2).reshape(128, 32)
    )
    rperm = np.ascontiguousarray(
        r.reshape(N_TILE, 128, 4).transpose(1, 0, 2).reshape(128, 32)
    )
    crt = np.ascontiguousarray(np.concatenate([cperm, rperm], axis=1)).astype(
        np.float32
    )
    return [
        {
            "wt": np.ascontiguousarray(
                W[k * ROWS : (k + 1) * ROWS].T
            ).reshape(N_TILE, 128, 4096),
            "crt": crt,
        }
        for k in range(N_CORES)
    ]


def kernel(orig_ub, orig_lb, prev_ub, prev_lb, alpha, W, b):
    orig_ub = np.asarray(orig_ub, dtype=np.float32)
    orig_lb = np.asarray(orig_lb, dtype=np.float32)
    prev_ub = np.asarray(prev_ub, dtype=np.float32)
    prev_lb = np.asarray(prev_lb, dtype=np.float32)
    alpha = np.asarray(alpha, dtype=np.float32)
    W = np.asarray(W, dtype=np.float32)
    b = np.asarray(b, dtype=np.float32)

    in_maps = _prep_in_maps(W, orig_ub, orig_lb)
    res = run_bass_kernel_spmd(_get_nc(), in_maps, list(range(N_CORES)))
    u1 = np.concatenate([res.results[k]["out"][0] for k in range(N_CORES)])
    u2 = np.concatenate([res.results[k]["out"][1] for k in range(N_CORES)])

    # epilogue: identical mask logic to the reference, in fp32 numpy
    neg = prev_ub <= 0.0
    cross = (prev_ub > 0.0) & (prev_lb < 0.0)
    denom = np.where(cross, prev_ub - prev_lb, np.float32(1.0)).astype(np.float32)
    ub_slope = np.where(
        cross, prev_ub / denom, np.where(neg, np.float32(0.0), np.float32(1.0))
    ).astype(np.float32)
    lb_slope = np.where(
        cross, alpha, np.where(neg, np.float32(0.0), np.float32(1.0))
    ).astype(np.float32)
    ub_bias = np.where(cross, -ub_slope * prev_lb, np.float32(0.0)).astype(np.float32)

    new_ub = ub_slope * (u1 + u2 + b) + ub_bias
    new_lb = lb_slope * (u1 - u2 + b)
    return np.stack([new_ub, new_lb]).astype(np.float32)
